# revision 1
# baseline (speedup 1.0000x reference)
"""DenseGATv2 layer on 8 Trainium2 NeuronCores (Bass/Tile).

Math: the reference computes, per head,
    e[i,j]  = leaky_relu(s_i[i] + s_j[j], 0.2)   (s_i = h@a_src, s_j = h@a_dst)
    attn    = softmax_j(where(adj[i,j], e, -9e15))
    out[i]  = attn @ h
Since exp is monotonic and softmax is scale-invariant per row i:
    exp(leaky_relu(s_i+s_j)) * exp(-0.2 s_i) = max(exp(s_j + 0.8 s_i), exp(0.2 s_j))
and the row-constant exp(-0.2 s_i) cancels in the softmax normalization.  With
per-node precomputes rep_i = exp(0.8 s_i) (replicated across partitions),
rv_j = exp(s_j) and v_j = exp(0.2 s_j) (per-partition scalars), the whole
masked softmax numerator for one (j-chunk, head) tile is:
    P'[j,i] = max(rep_i * rv_j, v_j)        one tensor_scalar   (bf16, 4x)
    Pm      = P' * mask[j,i]                one tensor_tensor   (bf16, 2x),
                                            4 heads stacked against a
                                            stride-0-repeat mask AP
— no dense exp/leaky passes on ScalarE at all.  An appended ones-column in the
aggregation operand yields the softmax denominator inside the same PE matmuls
that aggregate h (attention tile stationary, so the output lands
destination-rows-on-partitions and phase 2 is just reciprocal + scale).

Sharding: destination rows i split across 8 cores (512 rows each); every core
computes the full h = x @ [W | W@a_src | W@a_dst] locally (one 128-deep matmul
per j-chunk) and reduces over all 4096 source nodes j for its own rows.

Trn2 scheduling notes: walrus allows at most ONE hardware sync-wait per
engine instruction (extras must be legalized into EventSemaphore ops by
Bacc.finalize, which this kernel relies on).  To keep that legalization
cheap the kernel also ships all bulk inputs as a single concatenated
tensor (one DMA -> one queue semaphore) and drains h PSUM with one engine.
PSUM output accumulators are pre-zeroed with memset and accumulated with
start=False throughout: interleaved per-head accumulation regions sharing
a PSUM bank corrupt each other's first contribution when start=True zeroing
is used per region (observed on HW: last-written head exact, others short).
"""

import os

import numpy as np
import ml_dtypes

import concourse.bass as bass
import concourse.tile as tile
from concourse.bacc import Bacc
from concourse import mybir
from concourse.bass_utils import run_bass_kernel_spmd

bf16 = ml_dtypes.bfloat16

N, IN_DIM, HEADS, OUT_DIM = 4096, 128, 4, 64
NCORES, ROWS = 8, N // 8          # 512 dest rows per core
P = 128                           # partitions
C = N // P                        # 32 j-chunks
OWNC = ROWS // P                  # 4 own i-chunks per core
COLS = 2 * IN_DIM + 2 * HEADS     # 264 = 256 h cols + 4 s_src + 4 s_dst
DAUG = OUT_DIM + 1                # 65: head h-slice + ones column
BULK = ROWS + COLS + N            # xownT | W_aug | xT columns

_cache = {}


def _build_bass(repeat=1, hw_loop=False):
    nc = Bacc()
    f32 = mybir.dt.float32
    f16 = mybir.dt.float16
    bfl = mybir.dt.bfloat16
    Act = mybir.ActivationFunctionType
    Alu = mybir.AluOpType

    bulk = nc.declare_dram_parameter("bulk", [P, BULK], f32, isOutput=False)
    maskT = nc.declare_dram_parameter("maskT", [N, ROWS], bfl, isOutput=False)
    out = nc.declare_dram_parameter("out", [ROWS, HEADS * OUT_DIM], f32, isOutput=True)
    riT_dram = nc.dram_tensor("riT_scratch", [OWNC * HEADS, P], bfl)

    with tile.TileContext(nc) as tc:
        with (
            tc.tile_pool(name="consts", bufs=1) as consts,
            tc.tile_pool(name="hb", bufs=C) as hb_pool,
            tc.tile_pool(name="vr", bufs=C) as vr_pool,
            tc.tile_pool(name="mask", bufs=8) as mask_pool,
            tc.tile_pool(name="tt", bufs=4) as t_pool,
            tc.tile_pool(name="pm", bufs=4) as pm_pool,
            tc.tile_pool(name="fin", bufs=4) as fin_pool,
            tc.tile_pool(name="psout", bufs=1, space="PSUM") as ps_out_pool,
            tc.tile_pool(name="ps_h", bufs=3, space="PSUM") as ps_h_pool,
            tc.tile_pool(name="ps_s", bufs=1, space="PSUM") as ps_s_pool,
        ):
          import contextlib
          loop_ctx = (tc.For_i(0, repeat, 1,
                               hint_engines=tuple(mybir.EngineType(e) for e in
                                                  ("PE", "DVE", "Activation", "SP", "Pool")))
                      if hw_loop else contextlib.nullcontext())
          with loop_ctx:
           for _rep in range(1 if hw_loop else repeat):
            # per-own-chunk output accumulators: claim PSUM banks first so they
            # are never aliased with the h-matmul banks (no cross-pool WAW).
            ps_out = [ps_out_pool.tile([P, HEADS, DAUG], f32, tag=f"po{k}", name=f"ps_out{k}")
                      for k in range(OWNC)]
            for k in range(OWNC):
                nc.vector.memset(ps_out[k][:, :, :], 0.0)

            if os.environ.get("GAT_WARM", "1") == "1":
                # pre-warm the ACT exp table set while input DMAs run
                warm = consts.tile([1, 1], f32, tag="warm")
                nc.vector.memset(warm, 0.0)
                nc.scalar.activation(warm, warm, Act.Exp)

            # ---- all bulk inputs in ONE DMA -> one queue semaphore
            sb_bulk = consts.tile([P, BULK], f32, tag="sb_bulk")
            nc.sync.dma_start(out=sb_bulk[:, 0:ROWS + COLS], in_=bulk[:, 0:ROWS + COLS])
            nc.sync.dma_start(out=sb_bulk[:, ROWS + COLS:BULK], in_=bulk[:, ROWS + COLS:BULK])
            sb_xown = sb_bulk[:, 0:ROWS]
            sb_W = sb_bulk[:, ROWS:ROWS + COLS]
            sb_xT = sb_bulk[:, ROWS + COLS:BULK]
            w_sd = sb_bulk[:, ROWS + 2 * IN_DIM:ROWS + 2 * IN_DIM + HEADS]

            # ---- phase 0b: r_i = exp(0.8 s_src) for own rows, replicated
            # across partitions via DMA transpose + DRAM-bounce broadcast.
            ps_sown = ps_s_pool.tile([P, COLS], f32, tag="ps_s", name="ps_sown")
            for oc in range(OWNC):
                nc.tensor.matmul(
                    ps_sown[:, oc * HEADS:(oc + 1) * HEADS],
                    sb_xown[:, oc * P:(oc + 1) * P], w_sd,
                    start=True, stop=True,
                )
            vown = consts.tile([P, P], bfl, tag="vown")
            nc.vector.memset(vown, 0.0)
            nc.scalar.activation(vown[:, 0:OWNC * HEADS], ps_sown[:, 0:OWNC * HEADS],
                                 Act.Exp, scale=0.8)
            vT = consts.tile([P, P], bfl, tag="vT")
            nc.sync.dma_start(out=vT, in_=vown, transpose=True)
            nc.sync.dma_start(out=riT_dram[:, :], in_=vT[0:OWNC * HEADS, :])
            sb_rep = consts.tile([P, HEADS, ROWS], bfl, tag="sb_rep")
            base = riT_dram[:, :]
            if os.environ.get("GAT_BCAST", "new") == "new":
                for hd in range(HEADS):
                    bcast = bass.AP(tensor=base.tensor, offset=base.offset + hd * P,
                                    ap=[[0, P], [HEADS * P, OWNC], [1, P]])
                    nc.sync.dma_start(
                        out=sb_rep[:, hd, :].rearrange("p (oc t) -> p oc t", oc=OWNC),
                        in_=bcast)
            else:
                for hd in range(HEADS):
                    for oc in range(OWNC):
                        row = riT_dram[oc * HEADS + hd:oc * HEADS + hd + 1, :]
                        b = bass.AP(tensor=row.tensor, offset=row.offset,
                                    ap=[[0, P], row.ap[-1]])
                        nc.sync.dma_start(out=sb_rep[:, hd, oc * P:(oc + 1) * P], in_=b)

            # ---- phase 0c: h_aug per j-chunk; PSUM drained by VectorE only
            hb = []
            vr = []
            for c in range(C):
                ps_h = ps_h_pool.tile([P, COLS], f32, tag="ps_h")
                nc.tensor.matmul(ps_h, sb_xT[:, c * P:(c + 1) * P], sb_W,
                                 start=True, stop=True)
                hb_c = hb_pool.tile([P, HEADS, DAUG], bfl, tag="hb")
                nc.vector.memset(hb_c[:, :, OUT_DIM:DAUG], 1.0)
                nc.scalar.activation(
                    hb_c[:, :, 0:OUT_DIM],
                    ps_h[:, 0:2 * IN_DIM].rearrange("p (h d) -> p h d", h=HEADS),
                    Act.Copy,
                )
                vr_c = vr_pool.tile([P, 2, HEADS], f32, tag="vr")
                nc.scalar.activation(vr_c[:, 0, :], ps_h[:, 2 * IN_DIM + HEADS:COLS],
                                     Act.Exp, scale=0.2)
                nc.scalar.activation(vr_c[:, 1, :], ps_h[:, 2 * IN_DIM + HEADS:COLS],
                                     Act.Exp, scale=1.0)
                hb.append(hb_c)
                vr.append(vr_c)

            # ---- phase 1: hot loop over j-chunks
            for c in range(C):
                mask_c = mask_pool.tile([P, ROWS], bfl, tag="mask")
                nc.sync.dma_start(out=mask_c, in_=maskT[c * P:(c + 1) * P, :])
                t_all = t_pool.tile([P, HEADS, ROWS], bfl, tag="T")
                for hd in range(HEADS):
                    nc.vector.tensor_scalar(
                        out=t_all[:, hd, :], in0=sb_rep[:, hd, :],
                        scalar1=vr[c][:, 1, hd:hd + 1],
                        scalar2=vr[c][:, 0, hd:hd + 1],
                        op0=Alu.mult, op1=Alu.max,
                    )
                pm_all = pm_pool.tile([P, HEADS, ROWS], bfl, tag="pm")
                for hd in range(HEADS):
                    nc.vector.tensor_tensor(out=pm_all[:, hd, :],
                                            in0=t_all[:, hd, :], in1=mask_c,
                                            op=Alu.mult)
                for hd in range(HEADS):
                    for k in range(OWNC):
                        nc.tensor.matmul(
                            ps_out[k][:, hd, :],
                            pm_all[:, hd, k * P:(k + 1) * P], hb[c][:, hd, :],
                            start=False, stop=(c == C - 1),
                            skip_group_check=True,
                        )

            # ---- phase 2: normalize + store (dest rows already on partitions)
            for k in range(OWNC):
                out_k = fin_pool.tile([P, HEADS, OUT_DIM], f32, tag="outk")
                for hd in range(HEADS):
                    rcp = fin_pool.tile([P, 1], f32, tag="rcp")
                    nc.vector.reciprocal(rcp, ps_out[k][:, hd, OUT_DIM:DAUG])
                    if os.environ.get("GAT_FIN", "act") == "act":
                        nc.scalar.activation(
                            out_k[:, hd, :], ps_out[k][:, hd, 0:OUT_DIM],
                            Act.Copy, scale=rcp,
                        )
                    else:
                        nc.vector.tensor_scalar(
                            out=out_k[:, hd, :], in0=ps_out[k][:, hd, 0:OUT_DIM],
                            scalar1=rcp, scalar2=None, op0=Alu.mult,
                        )
                nc.sync.dma_start(
                    out=out[k * P:(k + 1) * P, :].rearrange("p (h d) -> p h d", h=HEADS),
                    in_=out_k,
                )
    nc.finalize()
    return nc


def _prep_in_maps(x, adj_mask, W_lin, a_src, a_dst):

    W_lin = np.asarray(W_lin, np.float32)
    W3 = W_lin.reshape(IN_DIM, HEADS, OUT_DIM).astype(np.float64)
    W_src = (W3 @ np.asarray(a_src, np.float64)).astype(np.float32)
    W_dst = (W3 @ np.asarray(a_dst, np.float64)).astype(np.float32)
    W_aug = np.concatenate([W_lin, W_src, W_dst], axis=1)
    x = np.asarray(x, np.float32)
    xT = np.ascontiguousarray(x.T)
    adj = np.asarray(adj_mask, bool)
    maskT = np.where(adj.T, np.float32(1.0), np.float32(0.0)).astype(bf16)

    in_maps = []
    for core in range(NCORES):
        sl = slice(core * ROWS, (core + 1) * ROWS)
        bulk = np.ascontiguousarray(
            np.concatenate([xT[:, sl], W_aug, xT], axis=1))
        in_maps.append({
            "bulk": bulk,
            "maskT": np.ascontiguousarray(maskT[:, sl]),
        })

    return in_maps


def kernel(x, adj_mask, W_lin, a_src, a_dst):
    if "nc" not in _cache:
        _cache["nc"] = _build_bass()
    nc = _cache["nc"]
    in_maps = _prep_in_maps(x, adj_mask, W_lin, a_src, a_dst)
    res = run_bass_kernel_spmd(nc, in_maps, core_ids=list(range(NCORES)))
    outs = [r["out"] for r in res.results]
    return np.concatenate(outs, axis=0).astype(np.float32)



# revision 3
# speedup vs baseline: 18.6795x; 18.6795x over previous
"""DenseGATv2 layer on 8 Trainium2 NeuronCores (Bass/Tile).

Math: the reference computes, per head,
    e[i,j]  = leaky_relu(s_i[i] + s_j[j], 0.2)   (s_i = h@a_src, s_j = h@a_dst)
    attn    = softmax_j(where(adj[i,j], e, -9e15))
    out[i]  = attn @ h
Since exp is monotonic and softmax is scale-invariant per row i:
    exp(leaky_relu(s_i+s_j)) * exp(-0.2 s_i) = max(exp(s_j + 0.8 s_i), exp(0.2 s_j))
and the row-constant exp(-0.2 s_i) cancels in the softmax normalization.  With
per-node precomputes rep_i = exp(0.8 s_i) (replicated across partitions),
rv_j = exp(s_j) and v_j = exp(0.2 s_j) (per-partition scalars), the masked
softmax numerator for one (j-chunk, head) tile is
    Pm[j,i] = max(rep_i * rv_j, v_j) * mask[j,i]
computed two ways to balance the DVE and ACT engines (DVE is the bottleneck):
  heads 0-1 (DVE): one tensor_scalar (mult+max, 4x mode) then one merged
      tensor_tensor against a stride-0-repeated mask AP (2x mode);
  heads 2-3 (ACT+DVE): t2 = Relu(rv_j*rep - v_j) on ScalarE (scale/bias are
      per-partition APs), then one fused scalar_tensor_tensor on DVE:
      Pm = (t2 + v_j) * mask   [max(a,b) = relu(a-b)+b].
All three exp vectors come from ONE ScalarE Exp over pre-scaled score columns
(host packs 0.8*W@a_src, W@a_dst, 0.2*W@a_dst into W_aug), so no dense
exp/leaky passes exist anywhere.  An appended ones-column in the aggregation
operand yields the softmax denominator inside the same PE matmuls that
aggregate h (attention tile stationary, so output lands dest-rows-on-
partitions and phase 2 is just reciprocal + scale).

Sharding: destination rows i split across 8 cores (512 rows each); every core
computes the full h = x @ W_aug locally (one 128-deep matmul per j-chunk) and
reduces over all 4096 source nodes j for its own rows.

Trn2 scheduling notes: walrus allows at most ONE hardware sync-wait per
engine instruction (extras are legalized into EventSemaphore ops by
Bacc.finalize, which this kernel relies on).  To keep that legalization
cheap the kernel ships all bulk inputs as one concatenated tensor (few DMAs
-> few queue semaphores), groups mask DMAs 4 j-chunks at a time, and drains
h PSUM with one engine.  PSUM output accumulators are pre-zeroed with memset
and accumulated with start=False throughout: interleaved per-head
accumulation regions sharing a PSUM bank corrupt each other's first
contribution when start=True zeroing is used per region (observed on HW).

repeat>1 (used only by the repeat-delta timing harness) runs the body
through a tc.For_i hardware loop so the NEFF stays the same size as the
repeat=1 build: the repeat-delta method assumes the per-call constant
(RPC + executable load, which grows with NEFF size) is identical for both
builds, which an unrolled body would break by ~R x.
"""

import os

import numpy as np
import ml_dtypes

import concourse.bass as bass
import concourse.tile as tile
from concourse.bacc import Bacc
from concourse import mybir
from concourse.bass_utils import run_bass_kernel_spmd

bf16 = ml_dtypes.bfloat16

N, IN_DIM, HEADS, OUT_DIM = 4096, 128, 4, 64
NCORES, ROWS = 8, N // 8          # 512 dest rows per core
P = 128                           # partitions
C = N // P                        # 32 j-chunks
GRP = 4                           # j-chunks per mask DMA
OWNC = ROWS // P                  # 4 own i-chunks per core
COLS = 2 * IN_DIM + 3 * HEADS     # 268 = 256 h cols + 4*(0.8s_src) + 4*s_dst + 4*(0.2s_dst)
DAUG = OUT_DIM + 1                # 65: head h-slice + ones column
BULK = ROWS + COLS + N            # xownT | W_aug | xT columns
NDVE = 2                          # heads on the pure-DVE path; HEADS-NDVE go ACT+STT

_cache = {}


def _build_bass(repeat=1, hw_loop=None):
    # repeat>1 is only used for repeat-delta timing; run the body through a
    # hardware loop so the NEFF (and its per-call load/ship cost, which the
    # repeat-delta method is supposed to cancel) stays the same size as the
    # repeat=1 build instead of growing ~linearly with the unroll factor.
    if hw_loop is None:
        hw_loop = repeat > 1 and os.environ.get("GAT_HWLOOP", "1") == "1"
    nc = Bacc()
    f32 = mybir.dt.float32
    bfl = mybir.dt.bfloat16
    Act = mybir.ActivationFunctionType
    Alu = mybir.AluOpType

    bulk = nc.declare_dram_parameter("bulk", [P, BULK], f32, isOutput=False)
    maskT = nc.declare_dram_parameter("maskT", [N, ROWS], bfl, isOutput=False)
    out = nc.declare_dram_parameter("out", [ROWS, HEADS * OUT_DIM], f32, isOutput=True)
    riT_dram = nc.dram_tensor("riT_scratch", [HEADS, ROWS], bfl)

    with tile.TileContext(nc) as tc:
        with (
            tc.tile_pool(name="consts", bufs=1) as consts,
            tc.tile_pool(name="hb", bufs=C) as hb_pool,
            tc.tile_pool(name="vr", bufs=C) as vr_pool,
            tc.tile_pool(name="mask", bufs=3) as mask_pool,
            tc.tile_pool(name="tt", bufs=4) as t_pool,
            tc.tile_pool(name="pm", bufs=4) as pm_pool,
            tc.tile_pool(name="fin", bufs=4) as fin_pool,
            tc.tile_pool(name="psout", bufs=1, space="PSUM") as ps_out_pool,
            tc.tile_pool(name="ps_h", bufs=3, space="PSUM") as ps_h_pool,
            tc.tile_pool(name="ps_s", bufs=1, space="PSUM") as ps_s_pool,
        ):
          import contextlib
          loop_ctx = (tc.For_i(0, repeat, 1,
                               hint_engines=tuple(mybir.EngineType(e) for e in
                                                  ("PE", "DVE", "Activation", "SP", "Pool")))
                      if hw_loop else contextlib.nullcontext())
          with loop_ctx:
           for _rep in range(1 if hw_loop else repeat):
            # per-own-chunk output accumulators: claim PSUM banks first so they
            # are never aliased with the h-matmul banks (no cross-pool WAW).
            ps_out = [ps_out_pool.tile([P, HEADS, DAUG], f32, tag=f"po{k}", name=f"ps_out{k}")
                      for k in range(OWNC)]
            for k in range(OWNC):
                nc.vector.memset(ps_out[k][:, :, :], 0.0)

            if os.environ.get("GAT_WARM", "1") == "1":
                # pre-warm the ACT exp table set while input DMAs run
                warm = consts.tile([1, 1], f32, tag="warm")
                nc.vector.memset(warm, 0.0)
                nc.scalar.activation(warm, warm, Act.Exp)

            # ---- bulk inputs: xownT+W first (feeds the rep_i chain), then xT
            # in quarters so the first h-matmuls don't wait on the whole 2MB.
            sb_bulk = consts.tile([P, BULK], f32, tag="sb_bulk")
            nc.sync.dma_start(out=sb_bulk[:, 0:ROWS + COLS], in_=bulk[:, 0:ROWS + COLS])
            XQ = N // 4
            for q in range(4):
                lo = ROWS + COLS + q * XQ
                nc.sync.dma_start(out=sb_bulk[:, lo:lo + XQ], in_=bulk[:, lo:lo + XQ])
            sb_xown = sb_bulk[:, 0:ROWS]
            sb_W = sb_bulk[:, ROWS:ROWS + COLS]
            sb_xT = sb_bulk[:, ROWS + COLS:BULK]
            w_s08 = sb_bulk[:, ROWS + 2 * IN_DIM:ROWS + 2 * IN_DIM + HEADS]

            # ---- phase 0b: rep_i = exp(0.8 s_src) for own rows, computed
            # directly transposed (heads on partitions, own-i on free dim) by
            # one matmul, then replicated across partitions via a DRAM-bounce
            # broadcast DMA.
            ps_sT = ps_s_pool.tile([HEADS, ROWS], f32, tag="ps_sT", name="ps_sT")
            nc.tensor.matmul(ps_sT, w_s08, sb_xown, start=True, stop=True)
            vownT = consts.tile([HEADS, ROWS], bfl, tag="vownT")
            nc.scalar.activation(vownT, ps_sT, Act.Exp)
            nc.sync.dma_start(out=riT_dram[:, :], in_=vownT)
            sb_rep = consts.tile([P, HEADS, ROWS], bfl, tag="sb_rep")
            base = riT_dram[:, :]
            bcast = bass.AP(tensor=base.tensor, offset=base.offset,
                            ap=[[0, P], [ROWS, HEADS], [1, ROWS]])
            nc.sync.dma_start(out=sb_rep, in_=bcast)

            # ---- phase 0c: h_aug per j-chunk; PSUM drained by ACT only.
            # One Exp over the pre-scaled score columns gives rv=exp(s_dst)
            # and v=exp(0.2 s_dst); one negated copy gives the Relu bias.
            hb = []
            vr = []
            vneg = []
            for c in range(C):
                ps_h = ps_h_pool.tile([P, COLS], f32, tag="ps_h")
                nc.tensor.matmul(ps_h, sb_xT[:, c * P:(c + 1) * P], sb_W,
                                 start=True, stop=True)
                hb_c = hb_pool.tile([P, HEADS, DAUG], bfl, tag="hb")
                nc.vector.memset(hb_c[:, :, OUT_DIM:DAUG], 1.0)
                nc.scalar.activation(
                    hb_c[:, :, 0:OUT_DIM],
                    ps_h[:, 0:2 * IN_DIM].rearrange("p (h d) -> p h d", h=HEADS),
                    Act.Copy,
                )
                vr_c = vr_pool.tile([P, 2 * HEADS], f32, tag="vr")
                nc.scalar.activation(vr_c, ps_h[:, 2 * IN_DIM + HEADS:COLS], Act.Exp)
                vneg_c = vr_pool.tile([P, HEADS], f32, tag="vneg")
                nc.scalar.activation(vneg_c, vr_c[:, HEADS:2 * HEADS],
                                     Act.Copy, scale=-1.0)
                hb.append(hb_c)
                vr.append(vr_c)
                vneg.append(vneg_c)

            # ---- phase 1: hot loop over j-chunks
            mask_g = None
            for c in range(C):
                g, b = divmod(c, GRP)
                if b == 0:
                    mask_g = mask_pool.tile([P, GRP, ROWS], bfl, tag="mask")
                    src = maskT[g * GRP * P:(g + 1) * GRP * P, :]
                    nc.sync.dma_start(
                        out=mask_g,
                        in_=src.rearrange("(grp p) i -> p grp i", p=P))
                mask_c = mask_g[:, b, :]
                rv = vr[c]
                pm_all = pm_pool.tile([P, HEADS, ROWS], bfl, tag="pm")

                # heads 0..NDVE-1: tensor_scalar max then merged mask multiply
                t01 = t_pool.tile([P, NDVE, ROWS], bfl, tag="T")
                for hd in range(NDVE):
                    nc.vector.tensor_scalar(
                        out=t01[:, hd, :], in0=sb_rep[:, hd, :],
                        scalar1=rv[:, hd:hd + 1],
                        scalar2=rv[:, HEADS + hd:HEADS + hd + 1],
                        op0=Alu.mult, op1=Alu.max,
                    )
                mask_rep = bass.AP(
                    tensor=mask_g.tensor,
                    offset=mask_g.offset + b * ROWS,
                    ap=[list(mask_g.ap[0]), [0, NDVE], [1, ROWS]],
                )
                nc.vector.tensor_tensor(out=pm_all[:, 0:NDVE, :], in0=t01,
                                        in1=mask_rep, op=Alu.mult)

                # heads NDVE..3: ScalarE relu-max, then fused (t2+v)*mask
                t23 = t_pool.tile([P, HEADS - NDVE, ROWS], bfl, tag="T2")
                for hd in range(NDVE, HEADS):
                    nc.scalar.activation(
                        t23[:, hd - NDVE, :], sb_rep[:, hd, :], Act.Relu,
                        scale=rv[:, hd:hd + 1],
                        bias=vneg[c][:, hd:hd + 1],
                    )
                    nc.vector.scalar_tensor_tensor(
                        out=pm_all[:, hd, :], in0=t23[:, hd - NDVE, :],
                        scalar=rv[:, HEADS + hd:HEADS + hd + 1],
                        in1=mask_c, op0=Alu.add, op1=Alu.mult,
                    )

                for hd in range(HEADS):
                    for k in range(OWNC):
                        nc.tensor.matmul(
                            ps_out[k][:, hd, :],
                            pm_all[:, hd, k * P:(k + 1) * P], hb[c][:, hd, :],
                            start=False, stop=(c == C - 1),
                            skip_group_check=True,
                        )

            # ---- phase 2: normalize + store (dest rows already on partitions)
            for k in range(OWNC):
                out_k = fin_pool.tile([P, HEADS, OUT_DIM], f32, tag="outk")
                rcp = fin_pool.tile([P, HEADS], f32, tag="rcp")
                nc.vector.reciprocal(
                    rcp, ps_out[k][:, :, OUT_DIM:DAUG].rearrange("p h one -> p (h one)"))
                for hd in range(HEADS):
                    if os.environ.get("GAT_FIN", "act") == "act":
                        nc.scalar.activation(
                            out_k[:, hd, :], ps_out[k][:, hd, 0:OUT_DIM],
                            Act.Copy, scale=rcp[:, hd:hd + 1],
                        )
                    else:
                        nc.vector.tensor_scalar(
                            out=out_k[:, hd, :], in0=ps_out[k][:, hd, 0:OUT_DIM],
                            scalar1=rcp[:, hd:hd + 1], scalar2=None, op0=Alu.mult,
                        )
                nc.sync.dma_start(
                    out=out[k * P:(k + 1) * P, :].rearrange("p (h d) -> p h d", h=HEADS),
                    in_=out_k,
                )
    nc.finalize()
    return nc


def _prep_in_maps(x, adj_mask, W_lin, a_src, a_dst):

    W_lin = np.asarray(W_lin, np.float32)
    W3 = W_lin.reshape(IN_DIM, HEADS, OUT_DIM).astype(np.float64)
    W_src = (W3 @ np.asarray(a_src, np.float64))
    W_dst = (W3 @ np.asarray(a_dst, np.float64))
    W_aug = np.concatenate(
        [W_lin,
         (0.8 * W_src).astype(np.float32),
         W_dst.astype(np.float32),
         (0.2 * W_dst).astype(np.float32)],
        axis=1)
    x = np.asarray(x, np.float32)
    xT = np.ascontiguousarray(x.T)
    adj = np.asarray(adj_mask, bool)
    maskT = np.where(adj.T, np.float32(1.0), np.float32(0.0)).astype(bf16)

    in_maps = []
    for core in range(NCORES):
        sl = slice(core * ROWS, (core + 1) * ROWS)
        bulk = np.ascontiguousarray(
            np.concatenate([xT[:, sl], W_aug, xT], axis=1))
        in_maps.append({
            "bulk": bulk,
            "maskT": np.ascontiguousarray(maskT[:, sl]),
        })

    return in_maps


def kernel(x, adj_mask, W_lin, a_src, a_dst):
    if "nc" not in _cache:
        _cache["nc"] = _build_bass()
    nc = _cache["nc"]
    in_maps = _prep_in_maps(x, adj_mask, W_lin, a_src, a_dst)
    res = run_bass_kernel_spmd(nc, in_maps, core_ids=list(range(NCORES)))
    outs = [r["out"] for r in res.results]
    return np.concatenate(outs, axis=0).astype(np.float32)


# revision 29
# speedup vs baseline: 1639.4374x; 87.7665x over previous
"""DenseGATv2 layer on 8 Trainium2 NeuronCores (Bass/Tile).

Math: the reference computes, per head,
    e[i,j]  = leaky_relu(s_i[i] + s_j[j], 0.2)   (s_i = h@a_src, s_j = h@a_dst)
    attn    = softmax_j(where(adj[i,j], e, -9e15))
    out[i]  = attn @ h
Since exp is monotonic and softmax is scale-invariant per row i:
    exp(leaky_relu(s_i+s_j)) * exp(-0.2 s_i) = max(exp(s_j + 0.8 s_i), exp(0.2 s_j))
and the row-constant exp(-0.2 s_i) cancels in the softmax normalization.  With
per-node precomputes rep_i = exp(0.8 s_i) (replicated across partitions),
rv_j = exp(s_j) and v_j = exp(0.2 s_j) (per-partition scalars), the masked
softmax numerator for one (j-chunk, head) tile is
    Pm[j,i] = max(rep_i * rv_j, v_j) * mask[j,i]
with one tensor_scalar per head (mult+max, 4x mode) and ONE tensor_tensor for
all 4 heads against a stride-0-repeated mask AP (2x mode, amortizes the
fixed ~58-cycle DVE issue bubble).  [An ACT-offload variant — Relu(rv*rep-v)
on ScalarE + fused scalar_tensor_tensor on DVE — was tried and measured
WORSE: scalar_tensor_tensor only has a 1x uop and ACT Relu gets no 2x accel.]
All three exp vectors come from ONE ScalarE Exp over pre-scaled score columns
(host packs 0.8*W@a_src, W@a_dst, 0.2*W@a_dst into W_aug), so no dense
exp/leaky passes exist anywhere.  An appended ones-column in the aggregation
operand yields the softmax denominator inside the same PE matmuls that
aggregate h (attention tile stationary, so output lands dest-rows-on-
partitions and phase 2 is just reciprocal + scale).

Sharding: destination rows i split across 8 cores (512 rows each); every core
computes the full h = x @ W_aug locally (one 128-deep matmul per j-chunk) and
reduces over all 4096 source nodes j for its own rows.

Trn2 scheduling notes: walrus allows at most ONE hardware sync-wait per
engine instruction (extras are legalized into EventSemaphore ops by
Bacc.finalize, which this kernel relies on).  DMA issues occupy ~1.6us of
issuing-queue time each, so they are split across two queues: SP owns the
rep_i critical chain (bf16 xownT|w08 side tensor -> s-matmul -> Exp ->
DRAM-bounce broadcast) plus W and the output stores, while the otherwise
idle GpSimd queue issues the xT quarters and grouped (4-chunk) mask DMAs,
first mask group first.  Everything that can be 16-bit is (x, W, mask, h,
attention tiles, output), which also halves host->HBM payload; PSUM stays
f32.  PSUM output accumulators are pre-zeroed with memset (in the startup
shadow) and accumulated with start=False throughout: interleaved per-head
accumulation regions sharing a PSUM bank corrupt each other's first
contribution when start=True zeroing is used per region (observed on HW).

repeat>1 (used only by the repeat-delta timing harness) runs the body
through a tc.For_i hardware loop so the NEFF stays the same size as the
repeat=1 build: the repeat-delta method assumes the per-call constant
(RPC + executable load, which grows with NEFF size) is identical for both
builds, which an unrolled body would break by ~R x.
"""

import os

import numpy as np
import ml_dtypes

import concourse.bass as bass
import concourse.tile as tile
from concourse.bacc import Bacc
from concourse import mybir
from concourse.bass_utils import run_bass_kernel_spmd

bf16 = ml_dtypes.bfloat16

N, IN_DIM, HEADS, OUT_DIM = 4096, 128, 4, 64
NCORES, ROWS = 8, N // 8          # 512 dest rows per core
P = 128                           # partitions
C = N // P                        # 32 j-chunks
GRP = 4                           # j-chunks per mask DMA
OWNC = ROWS // P                  # 4 own i-chunks per core
COLS = 2 * IN_DIM + 2 * HEADS     # 264 = 256 h cols + 4*s_dst + 4*(0.2s_dst)
DAUG = OUT_DIM + 1                # 65: head h-slice + ones column
BULK = COLS + N                   # W_aug | xT columns (f32)
XB = ROWS + HEADS                 # xownT | 0.8*W_src columns (bf16 side channel)

_cache = {}


def _build_bass(repeat=1, hw_loop=None):
    # repeat>1 is only used for repeat-delta timing; run the body through a
    # hardware loop so the NEFF (and its per-call load/ship cost, which the
    # repeat-delta method is supposed to cancel) stays the same size as the
    # repeat=1 build instead of growing ~linearly with the unroll factor.
    if hw_loop is None:
        hw_loop = repeat > 1 and os.environ.get("GAT_HWLOOP", "1") == "1"
    nc = Bacc()
    f32 = mybir.dt.float32
    bfl = mybir.dt.bfloat16
    Act = mybir.ActivationFunctionType
    Alu = mybir.AluOpType

    bulk = nc.declare_dram_parameter("bulk", [P, BULK], bfl, isOutput=False)
    xb = nc.declare_dram_parameter("xb", [P, XB], bfl, isOutput=False)
    maskT = nc.declare_dram_parameter("maskT", [N, ROWS], bfl, isOutput=False)
    out = nc.declare_dram_parameter("out", [ROWS, HEADS * OUT_DIM], bfl, isOutput=True)
    riT_dram = nc.dram_tensor("riT_scratch", [HEADS, ROWS], bfl)

    with tile.TileContext(nc) as tc:
        with (
            tc.tile_pool(name="consts", bufs=1) as consts,
            tc.tile_pool(name="hb", bufs=C) as hb_pool,
            tc.tile_pool(name="vr", bufs=C) as vr_pool,
            tc.tile_pool(name="mask", bufs=3) as mask_pool,
            tc.tile_pool(name="tt", bufs=4) as t_pool,
            tc.tile_pool(name="pm", bufs=4) as pm_pool,
            tc.tile_pool(name="fin", bufs=4) as fin_pool,
            tc.tile_pool(name="psout", bufs=1, space="PSUM") as ps_out_pool,
            tc.tile_pool(name="ps_h", bufs=3, space="PSUM") as ps_h_pool,
            tc.tile_pool(name="ps_s", bufs=1, space="PSUM") as ps_s_pool,
        ):
          import contextlib
          loop_ctx = (tc.For_i(0, repeat, 1,
                               hint_engines=tuple(mybir.EngineType(e) for e in
                                                  ("PE", "DVE", "Activation", "SP", "Pool")))
                      if hw_loop else contextlib.nullcontext())
          with loop_ctx:
           for _rep in range(1 if hw_loop else repeat):
            # per-own-chunk output accumulators: claim PSUM banks first so they
            # are never aliased with the h-matmul banks (no cross-pool WAW).
            ps_out = [ps_out_pool.tile([P, HEADS, DAUG], f32, tag=f"po{k}", name=f"ps_out{k}")
                      for k in range(OWNC)]
            for k in range(OWNC):
                nc.vector.memset(ps_out[k][:, :, :], 0.0)

            if os.environ.get("GAT_WARM", "1") == "1":
                # pre-warm the ACT exp table set while input DMAs run
                warm = consts.tile([1, 1], f32, tag="warm")
                nc.vector.memset(warm, 0.0)
                nc.scalar.activation(warm, warm, Act.Exp)

            # ---- bulk inputs.  DMA *issue* costs ~1.6us of queue time each,
            # so the issues are spread over two engines: SP carries only the
            # rep_i critical chain (bf16 xownT+w08 side channel -> riT ->
            # sb_rep, in program order so nothing queues ahead of it) and the
            # W columns; the idle GpSimd queue issues the xT quarters and all
            # mask groups (first mask group ahead of the xT quarters, since
            # the first chunk's mask multiply needs it ~5us in).
            sb_bulk = consts.tile([P, BULK], bfl, tag="sb_bulk")
            sb_xb = consts.tile([P, XB], bfl, tag="sb_xb")
            nc.sync.dma_start(out=sb_xb, in_=xb[:, :])
            nc.sync.dma_start(out=sb_bulk[:, 0:COLS], in_=bulk[:, 0:COLS])
            sb_xown = sb_xb[:, 0:ROWS]
            w_s08 = sb_xb[:, ROWS:ROWS + HEADS]
            sb_W = sb_bulk[:, 0:COLS]
            sb_xT = sb_bulk[:, COLS:BULK]

            XQ = N // 4
            mask_gs = []

            def mask_dma(g):
                mg = mask_pool.tile([P, GRP, ROWS], bfl, tag="mask")
                msrc = maskT[g * GRP * P:(g + 1) * GRP * P, :]
                nc.gpsimd.dma_start(
                    out=mg, in_=msrc.rearrange("(grp p) i -> p grp i", p=P))
                mask_gs.append(mg)

            mask_dma(0)
            nc.gpsimd.dma_start(out=sb_bulk[:, COLS:COLS + XQ],
                                in_=bulk[:, COLS:COLS + XQ])
            mask_dma(1)
            for q in range(1, 4):
                lo = COLS + q * XQ
                nc.gpsimd.dma_start(out=sb_bulk[:, lo:lo + XQ], in_=bulk[:, lo:lo + XQ])
            for g in range(2, C // GRP):
                mask_dma(g)

            # ---- phase 0b: rep_i = exp(0.8 s_src) for own rows, computed
            # directly transposed (heads on partitions, own-i on free dim) by
            # one matmul, then replicated across partitions via a DRAM-bounce
            # broadcast DMA.
            ps_sT = ps_s_pool.tile([HEADS, ROWS], f32, tag="ps_sT", name="ps_sT")
            nc.tensor.matmul(ps_sT, w_s08, sb_xown, start=True, stop=True)
            vownT = consts.tile([HEADS, ROWS], bfl, tag="vownT")
            nc.scalar.activation(vownT, ps_sT, Act.Exp)
            nc.sync.dma_start(out=riT_dram[:, :], in_=vownT)
            sb_rep = consts.tile([P, HEADS, ROWS], bfl, tag="sb_rep")
            base = riT_dram[:, :]
            bcast = bass.AP(tensor=base.tensor, offset=base.offset,
                            ap=[[0, P], [ROWS, HEADS], [1, ROWS]])
            nc.sync.dma_start(out=sb_rep, in_=bcast)

            # ---- phase 0c: h_aug per j-chunk; PSUM drained by ACT only.
            # One Exp over the pre-scaled score columns gives rv=exp(s_dst)
            # and v=exp(0.2 s_dst).  The ones column is written by ACT too
            # (Copy with scale=0, bias=1 skips the input read) to keep the
            # bottleneck DVE engine free of per-chunk bookkeeping.
            hb = []
            vr = []
            for c in range(C):
                ps_h = ps_h_pool.tile([P, COLS], f32, tag="ps_h")
                nc.tensor.matmul(ps_h, sb_xT[:, c * P:(c + 1) * P], sb_W,
                                 start=True, stop=True)
                hb_c = hb_pool.tile([P, HEADS, DAUG], bfl, tag="hb")
                nc.scalar.activation(hb_c[:, :, OUT_DIM:DAUG],
                                     ps_h[:, 0:HEADS].rearrange("p (h one) -> p h one", h=HEADS),
                                     Act.Copy, scale=0.0, bias=1.0)
                nc.scalar.activation(
                    hb_c[:, :, 0:OUT_DIM],
                    ps_h[:, 0:2 * IN_DIM].rearrange("p (h d) -> p h d", h=HEADS),
                    Act.Copy,
                )
                vr_c = vr_pool.tile([P, 2 * HEADS], f32, tag="vr")
                nc.scalar.activation(vr_c, ps_h[:, 2 * IN_DIM:COLS], Act.Exp)
                hb.append(hb_c)
                vr.append(vr_c)

            # ---- phase 1: hot loop over j-chunks
            for c in range(C):
                g, b = divmod(c, GRP)
                mask_g = mask_gs[g]
                rv = vr[c]
                pm_all = pm_pool.tile([P, HEADS, ROWS], bfl, tag="pm")

                # per-head tensor_scalar max (4x mode), then ONE merged mask
                # multiply for all heads (2x mode, stride-0-repeated mask AP)
                t_all = t_pool.tile([P, HEADS, ROWS], bfl, tag="T")
                for hd in range(HEADS):
                    nc.vector.tensor_scalar(
                        out=t_all[:, hd, :], in0=sb_rep[:, hd, :],
                        scalar1=rv[:, hd:hd + 1],
                        scalar2=rv[:, HEADS + hd:HEADS + hd + 1],
                        op0=Alu.mult, op1=Alu.max,
                    )
                mask_rep = bass.AP(
                    tensor=mask_g.tensor,
                    offset=mask_g.offset + b * ROWS,
                    ap=[list(mask_g.ap[0]), [0, HEADS], [1, ROWS]],
                )
                nc.vector.tensor_tensor(out=pm_all, in0=t_all,
                                        in1=mask_rep, op=Alu.mult)

                # k-major so on the last chunk each ps_out[k] closes early,
                # letting its reciprocal+scale overlap the remaining matmuls
                for k in range(OWNC):
                    for hd in range(HEADS):
                        nc.tensor.matmul(
                            ps_out[k][:, hd, :],
                            pm_all[:, hd, k * P:(k + 1) * P], hb[c][:, hd, :],
                            start=False, stop=(c == C - 1),
                            skip_group_check=True,
                        )

            # ---- phase 2: normalize + store (dest rows already on partitions).
            # k-major matmul order above means ps_out[k] closes early; scale
            # work is split DVE/ACT per head-half and each k's output row
            # block DMAs out as soon as its scales land (SP queue is idle by
            # now, so the four issues pipeline with the remaining scales).
            for k in range(OWNC):
                fin = fin_pool.tile([P, HEADS, OUT_DIM], bfl, tag="fin")
                rcp = fin_pool.tile([P, HEADS], f32, tag="rcp")
                nc.vector.reciprocal(
                    rcp, ps_out[k][:, :, OUT_DIM:DAUG].rearrange("p h one -> p (h one)"))
                for hd in range(HEADS):
                    if hd < HEADS // 2:
                        nc.scalar.activation(
                            fin[:, hd, :], ps_out[k][:, hd, 0:OUT_DIM],
                            Act.Copy, scale=rcp[:, hd:hd + 1],
                        )
                    else:
                        nc.vector.tensor_scalar(
                            out=fin[:, hd, :], in0=ps_out[k][:, hd, 0:OUT_DIM],
                            scalar1=rcp[:, hd:hd + 1], scalar2=None, op0=Alu.mult,
                        )
                nc.sync.dma_start(
                    out=out[k * P:(k + 1) * P, :].rearrange("p (h d) -> p h d", h=HEADS),
                    in_=fin,
                )
    nc.finalize()
    return nc


def _prep_in_maps(x, adj_mask, W_lin, a_src, a_dst):

    W_lin = np.asarray(W_lin, np.float32)
    W3 = W_lin.reshape(IN_DIM, HEADS, OUT_DIM).astype(np.float64)
    W_src = (W3 @ np.asarray(a_src, np.float64))
    W_dst = (W3 @ np.asarray(a_dst, np.float64))
    W_aug = np.concatenate(
        [W_lin,
         W_dst.astype(np.float32),
         (0.2 * W_dst).astype(np.float32)],
        axis=1)
    w_s08 = (0.8 * W_src).astype(np.float32)
    x = np.asarray(x, np.float32)
    xT = np.ascontiguousarray(x.T)
    adj = np.asarray(adj_mask, bool)
    maskT = np.where(adj.T, np.float32(1.0), np.float32(0.0)).astype(bf16)

    in_maps = []
    for core in range(NCORES):
        sl = slice(core * ROWS, (core + 1) * ROWS)
        bulk = np.ascontiguousarray(np.concatenate([W_aug, xT], axis=1)).astype(bf16)
        xb = np.ascontiguousarray(
            np.concatenate([xT[:, sl], w_s08], axis=1)).astype(bf16)
        in_maps.append({
            "bulk": bulk,
            "xb": xb,
            "maskT": np.ascontiguousarray(maskT[:, sl]),
        })

    return in_maps


def kernel(x, adj_mask, W_lin, a_src, a_dst):
    if "nc" not in _cache:
        _cache["nc"] = _build_bass()
    nc = _cache["nc"]
    in_maps = _prep_in_maps(x, adj_mask, W_lin, a_src, a_dst)
    res = run_bass_kernel_spmd(nc, in_maps, core_ids=list(range(NCORES)))
    outs = [r["out"] for r in res.results]
    return np.concatenate(outs, axis=0).astype(np.float32)
